# revision 28
# baseline (speedup 1.0000x reference)
"""Trainium2 Bass kernel for nn_DYConv_2d (dynamic-kernel CNN, 4 DYConv
stages + triplet attention gate head), data-parallel over batch across 8
NeuronCores.

v2 strategy (vs v1 baseline at ~860us):
 - per-sample pipelining across stage boundaries: the attention MLP runs in
   3 sub-batches (2/2/4 samples) so sample 0's convs start right after its
   BN apply instead of after the whole batch.
 - softmax exp via 2nd-order Taylor on VectorE (|logit/34| < 0.01), so the
   ScalarE activation table never thrashes between exp and sqrt sets.
 - convs: 9 shifted accumulating matmuls into multi-bank PSUM tiles
   (group A: 3 banks, group B: <=2), ONE strided eviction activation per
   group (bias + accum_out sums) instead of per-chunk evictions.
 - sum-of-squares via VectorE/GpSimd tensor ops on the bf16 conv output.
 - BN applies on ScalarE with fused pooled accum_out, writing per-MLP-batch
   pooled tiles so downstream MLPs unblock per batch.
 - gate head fully batched: one y4 tile [64, 8*1936]; 4 batched plane
   reductions (VectorE); channel max via one GpSimd partition reduce;
   channel sum via K=64 ones-matmuls + ScalarE evictions; band convs as 14
   matmuls per gate over all 8 samples (3D APs, unpadded edge slicing).
"""
import numpy as np

import concourse.bass as bass
import concourse.bacc as bacc
import concourse.bass_isa as bass_isa
import concourse.mybir as mybir
import concourse.tile as tile
from concourse.bass_utils import run_bass_kernel_spmd

N_CORES = 8
S = 8  # samples per core
TEMP = 34.0
EPS = 1e-5
FP = mybir.dt.float32
BF = mybir.dt.bfloat16
AF = mybir.ActivationFunctionType
ALU = mybir.AluOpType
AX = mybir.AxisListType

# (cin, cout, pad, Hin, Hout, hid)
STAGES = [
    (100, 60, 1, 48, 48, 26),
    (60, 120, 1, 48, 48, 16),
    (120, 120, 0, 48, 46, 31),
    (120, 64, 0, 46, 44, 31),
]
# conv row-chunk plans: (groupA, groupB); each group is a list of
# (y0, rows) with uniform rows within a group (except stage3 B).
CHUNKS = {
    1: ([(0, 10), (10, 10), (20, 10)], [(30, 9), (39, 9)]),
    2: ([(0, 10), (10, 10), (20, 10)], [(30, 9), (39, 9)]),
    3: ([(0, 10), (10, 10), (20, 10)], [(30, 10), (40, 6)]),
    4: ([(0, 11), (11, 11), (22, 11)], [(33, 11)]),
}
BATCHES = [(0, 2), (2, 4), (4, 8)]
H4 = 44  # final spatial
HW4 = H4 * H4
NB = 64  # full batch
PW = 52  # padded plane width (left pad 2 keeps rows 4B aligned)


def build_nc():
    nc = bacc.Bacc(
        "TRN2",
        target_bir_lowering=False,
        debug=False,
        enable_asserts=True,
        num_devices=N_CORES,
    )
    # ---- DRAM parameters -------------------------------------------------
    xin = nc.dram_tensor("x", [S, 100, 50 * PW], BF, kind="ExternalInput")
    wt_d, wb_d, a1_d, a2_d, bng_d, bnb_d = {}, {}, {}, {}, {}, {}
    for i, (cin, cout, pad, hin, hout, hid) in enumerate(STAGES, 1):
        wt_d[i] = nc.dram_tensor(f"wt{i}", [cin, 36 * cout], BF, kind="ExternalInput")
        wb_d[i] = nc.dram_tensor(f"wb{i}", [4, cout], FP, kind="ExternalInput")
        a1_d[i] = nc.dram_tensor(f"a1w{i}", [cin, hid], FP, kind="ExternalInput")
        a2_d[i] = nc.dram_tensor(f"a2w{i}", [hid + 1, 4], FP, kind="ExternalInput")
        bng_d[i] = nc.dram_tensor(f"bng{i}", [cout, 1], FP, kind="ExternalInput")
        bnb_d[i] = nc.dram_tensor(f"bnb{i}", [cout, 1], FP, kind="ExternalInput")
    fc3w_d = nc.dram_tensor("fc3w", [100, 64], FP, kind="ExternalInput")
    fc3b_d = nc.dram_tensor("fc3b", [S, 64], FP, kind="ExternalInput")
    gb_d = [
        nc.dram_tensor("gb0", [64, 14 * 64], BF, kind="ExternalInput"),
        nc.dram_tensor("gb1", [64, 14 * 64], BF, kind="ExternalInput"),
        nc.dram_tensor("gb2", [44, 14 * 44], BF, kind="ExternalInput"),
    ]
    gbn_d = nc.dram_tensor("gbn", [1, 6], FP, kind="ExternalInput")
    ident_d = nc.dram_tensor("ident", [16, 16], FP, kind="ExternalInput")

    x1o = nc.dram_tensor("x1o", [S, 64], FP, kind="ExternalOutput")
    o1o = nc.dram_tensor("o1o", [64, S], FP, kind="ExternalOutput")

    with tile.TileContext(nc) as tc:
        V, A, G = nc.vector, nc.scalar, nc.gpsimd
        from contextlib import ExitStack

        est = ExitStack()
        pact = est.enter_context(tc.tile_pool(name="pact", bufs=1))
        psm = est.enter_context(tc.tile_pool(name="psm", bufs=1))
        pc = est.enter_context(tc.tile_pool(name="pc", bufs=1))
        pdram = est.enter_context(tc.tile_pool(name="pdram", bufs=1, space="DRAM"))
        pwt_cm = tc.tile_pool(name="pwt", bufs=1)
        pwt = pwt_cm.__enter__()
        pz_cm = tc.tile_pool(name="pz", bufs=1)
        pz = pz_cm.__enter__()

        dma_engines = [nc.sync, nc.scalar, nc.gpsimd]
        dma_rr = [0]

        def dma(dst, src):
            eng = dma_engines[dma_rr[0] % len(dma_engines)]
            dma_rr[0] += 1
            eng.dma_start(out=dst, in_=src)

        # ---- load x first (feeds the critical path) --------------------
        cur_tiles = []
        for b in range(S):
            xt = pact.tile([120, 50 * PW], BF, tag="xt", bufs=S)
            dma(xt[0:100, :], xin[b, :, :])
            cur_tiles.append(xt)

        # ---- constants -------------------------------------------------
        wt_t, wb_t, a1_t, a2_t, bng_t, bnb_t = {}, {}, {}, {}, {}, {}
        for i, (cin, cout, pad, hin, hout, hid) in enumerate(STAGES, 1):
            wt_t[i] = pwt.tile([cin, 36 * cout], BF, tag=f"wt{i}", name=f"wt{i}")
            dma(wt_t[i][:], wt_d[i][:, :])
            wb_t[i] = pc.tile([4, cout], FP, tag=f"wb{i}", name=f"wb{i}")
            dma(wb_t[i][:], wb_d[i][:, :])
            a1_t[i] = pc.tile([cin, hid], FP, tag=f"a1w{i}", name=f"a1w{i}")
            dma(a1_t[i][:], a1_d[i][:, :])
            a2_t[i] = pc.tile([hid + 1, 4], FP, tag=f"a2w{i}", name=f"a2w{i}")
            dma(a2_t[i][:], a2_d[i][:, :])
            bng_t[i] = pc.tile([cout, 1], FP, tag=f"bng{i}", name=f"bng{i}")
            dma(bng_t[i][:], bng_d[i][:, :])
            bnb_t[i] = pc.tile([cout, 1], FP, tag=f"bnb{i}", name=f"bnb{i}")
            dma(bnb_t[i][:], bnb_d[i][:, :])
        fc3w_t = pc.tile([100, 64], FP, tag="fc3w")
        dma(fc3w_t[:], fc3w_d[:, :])
        fc3b_t = pc.tile([S, 64], FP, tag="fc3b")
        dma(fc3b_t[:], fc3b_d[:, :])
        gb_t = []
        for g, kk in enumerate((64, 64, 44)):
            tb = pc.tile([kk, 14 * kk], BF, tag=f"gb{g}", name=f"gb{g}")
            dma(tb[:], gb_d[g][:, :])
            gb_t.append(tb)
        gbn_t = pc.tile([1, 6], FP, tag="gbn")
        dma(gbn_t[:], gbn_d[:, :])
        ident_t = pc.tile([16, 16], FP, tag="ident")
        dma(ident_t[:], ident_d[:, :])
        ones_row = pc.tile([1, 128], FP, tag="ones_row")
        V.memset(ones_row[:], 1.0)
        ones_row_bf = pc.tile([1, 128], BF, tag="ones_row_bf")
        V.memset(ones_row_bf[:], 1.0)
        ones_col = pc.tile([128, 1], BF, tag="ones_col")
        V.memset(ones_col[:], 1.0)
        ones_colf = pc.tile([128, 1], FP, tag="ones_colf")
        V.memset(ones_colf[:], 1.0)
        eps_col = pc.tile([128, 1], FP, tag="eps_col")
        V.memset(eps_col[:], EPS)

        # persistent DMA-written tiles (virgin SBUF; see baseline note on
        # DMA-after-DMA slot reuse)
        af32s = {}
        for i in range(1, 5):
            for bi in range(3):
                af32s[(i, bi)] = psm.tile(
                    [1, 4 * (BATCHES[bi][1] - BATCHES[bi][0])], FP,
                    tag=f"af32_{i}_{bi}", name=f"af32_{i}_{bi}")
        msums = [psm.tile([120, 1], FP, tag=f"msum{j}", name=f"msum{j}")
                 for j in range(2)]  # noqa
        hTs = {}
        for i in range(1, 5):
            for bi in range(3):
                hTs[(i, bi)] = psm.tile([32, 8], FP, tag=f"hT_{i}_{bi}",
                                        name=f"hT_{i}_{bi}")
        msqs = [psm.tile([120, 1], FP, tag=f"msq{j}", name=f"msq{j}")
                for j in range(2)]
        gtot_in = psm.tile([1, 6], FP, tag="gtot_in")
        g3max_bt = [pact.tile([44, 50], BF, tag=f"g3max{b}", name=f"g3max{b}")
                    for b in range(S)]
        g3sum_bt = [pact.tile([44, 50], BF, tag=f"g3sum{b}", name=f"g3sum{b}")
                    for b in range(S)]
        for b in range(S):
            V.memset(g3max_bt[b][:], 0.0)
            V.memset(g3sum_bt[b][:], 0.0)
        s3rows = [psm.tile([1, HW4], BF, tag=f"s3row{j}", name=f"s3row{j}")
                  for j in range(2)]

        # big persistent activations
        y4all = pact.tile([64, S * HW4], BF, tag="y4all", name="y4all")
        trashV = pact.tile([124, 2304], BF, tag="trashV", name="trashV")
        trashG = pact.tile([124, 2304], BF, tag="trashG", name="trashG")

        stage_ps_cm = tc.tile_pool(name="stageps", bufs=1, space="PSUM")
        stage_ps = stage_ps_cm.__enter__()
        conv_ps_cm = tc.tile_pool(name="convps", bufs=1, space="PSUM")
        conv_ps = conv_ps_cm.__enter__()

        # ================= phase A: pooled, x1 =================
        # after start-up, route DMAs through the sync queue only
        del dma_engines[1:]

        # per-MLP-batch pooled tiles (stage-1 attention inputs)
        pooled_b = [psm.tile([124, b1 - b0], FP, tag="pooled", bufs=6,
                             name=f"pooled_a{bi}")
                    for bi, (b0, b1) in enumerate(BATCHES)]
        for b in range(S):
            xv = cur_tiles[b][0:100, :].rearrange("p (h w) -> p h w", h=50)
            view = xv[:, 1:49, 2:50]
            bi = 0 if b < 2 else (1 if b < 4 else 2)
            col = b - BATCHES[bi][0]
            dstc = pooled_b[bi][0:100, col:col + 1]
            if b < 4:
                V.tensor_reduce(dstc, view, axis=AX.XY, op=ALU.add)
            else:
                A.activation(trashV[0:100, 0:2304], view, AF.Copy,
                             accum_out=dstc)

        # x1 = pooled/HW @ fc3w.T + fc3b (per batch; fc3w pre-scaled on host)
        pooledcat = psm.tile([100, 8], FP, tag="pooledcat")
        for bi, (b0, b1) in enumerate(BATCHES):
            V.tensor_copy(pooledcat[:, b0:b1], pooled_b[bi][0:100, :])
        ps_x1 = stage_ps.tile([124, 64], FP, tag="smallps", bufs=2)
        nc.tensor.matmul(ps_x1[:S, :64], pooledcat[:], fc3w_t[:],
                         start=True, stop=True)
        x1sb = psm.tile([8, 64], FP, tag="x1sb")
        V.tensor_tensor(x1sb[:], ps_x1[:S, :64], fc3b_t[:], op=ALU.add)
        dma(x1o[:, :], x1sb[:])

        # ================= 4 DYConv stages =================
        zts = {}

        def emit_mlp(i, bi, pooled_tile):
            """Attention MLP + Taylor softmax for samples [b0, b1) of stage
            i. Produces aggbT col block and bcs [cin, 4*nb]."""
            cin, cout, pad, hin, hout, hid = STAGES[i - 1]
            b0, b1 = BATCHES[bi]
            nb = b1 - b0
            ps_h = stage_ps.tile([124, 64], FP, tag="smallps", bufs=2)
            nc.tensor.matmul(ps_h[:hid, :nb], a1_t[i][:], pooled_tile[:cin, :],
                             start=True, stop=True)
            hT = hTs[(i, bi)]
            dma(hT[hid:hid + 1, 0:nb], ones_row[:, 0:nb])
            V.tensor_scalar(hT[:hid, 0:nb], ps_h[:hid, :nb], 0.0, None,
                            op0=ALU.max)
            ps_l = stage_ps.tile([124, 64], FP, tag="smallps", bufs=2)
            nc.tensor.matmul(ps_l[:nb, 0:4], hT[:hid + 1, 0:nb], a2_t[i][:],
                             start=True, stop=True)
            # softmax((l)/T) with 2nd-order Taylor exp: q = y + y^2/2,
            # attn = (1+q)/(4+sum q) computed as q*r + r.
            tq = psm.tile([8, 4], FP, tag="tq", bufs=2)
            V.tensor_scalar(tq[:nb, :], ps_l[:nb, 0:4], 0.5 / TEMP, 1.0,
                            op0=ALU.mult, op1=ALU.add)
            q = psm.tile([8, 4], FP, tag="q", bufs=2)
            V.scalar_tensor_tensor(q[:nb, :], ps_l[:nb, 0:4], 1.0 / TEMP,
                                   tq[:nb, :], op0=ALU.mult, op1=ALU.mult)
            qs = psm.tile([8, 1], FP, tag="qs", bufs=2)
            V.tensor_reduce(qs[:nb, :], q[:nb, :], axis=AX.X, op=ALU.add)
            d4 = psm.tile([8, 1], FP, tag="d4", bufs=2)
            V.tensor_scalar(d4[:nb, :], qs[:nb, :], 4.0, None, op0=ALU.add)
            rc = psm.tile([8, 1], FP, tag="rc", bufs=2)
            V.reciprocal(rc[:nb, :], d4[:nb, :])
            attn = psm.tile([8, 4], FP, tag="attn", bufs=2)
            V.tensor_scalar(attn[:nb, :], q[:nb, :], rc[:nb, :], rc[:nb, :],
                            op0=ALU.mult, op1=ALU.add)
            # aggregated bias: aggbT = wb.T @ attn.T  [cout, nb]
            ps_aT = stage_ps.tile([124, 64], FP, tag="smallps", bufs=2)
            nc.tensor.transpose(ps_aT[0:4, :nb], attn[:nb, :],
                                ident_t[0:nb, 0:nb])
            attnT = psm.tile([4, 8], FP, tag="attnT", bufs=2)
            A.activation(attnT[:, 0:nb], ps_aT[0:4, :nb], AF.Copy)
            ps_ab = stage_ps.tile([124, 64], FP, tag="smallps", bufs=2)
            nc.tensor.matmul(ps_ab[:cout, :nb], wb_t[i][:], attnT[:, 0:nb],
                             start=True, stop=True)
            aggb = psm.tile([124, 8], FP, tag="aggb", bufs=2)
            A.activation(aggb[:cout, 0:nb], ps_ab[:cout, :nb], AF.Copy)
            # broadcast attn down cin partitions: bcs [cin, 4*nb]
            af32 = af32s[(i, bi)]
            dma(af32[:], attn[:nb, :])
            ps_bc = stage_ps.tile([124, 64], FP, tag="smallps", bufs=2)
            nc.tensor.matmul(ps_bc[:cin, :4 * nb], ones_row[0:1, 0:cin],
                             af32[:], start=True, stop=True)
            bcs = psm.tile([124, 16], FP, tag="bcs", bufs=2)
            A.activation(bcs[:cin, 0:4 * nb], ps_bc[:cin, :4 * nb], AF.Copy)
            return aggb, bcs

        def emit_sample(i, b, bi, aggb, bcs, sums, sqs):
            """Weight aggregation + conv + eviction + sumsq for sample b."""
            cin, cout, pad, hin, hout, hid = STAGES[i - 1]
            wout = hout
            col = b - BATCHES[bi][0]
            wtv = wt_t[i][:].rearrange("p (k t o) -> p k t o", k=4, t=9)
            agA = pz.tile([120, 9 * 120], BF, tag="agA", bufs=2)
            agB = pz.tile([120, 9 * 120], BF, tag="agB", bufs=2)
            a_ = agA[0:cin, 0:9 * cout]
            b_ = agB[0:cin, 0:9 * cout]
            V.tensor_scalar(a_, wtv[:, 0, :, :], bcs[:cin, 4 * col:4 * col + 1],
                            None, op0=ALU.mult)
            V.scalar_tensor_tensor(b_, wtv[:, 1, :, :],
                                   bcs[:cin, 4 * col + 1:4 * col + 2], a_,
                                   op0=ALU.mult, op1=ALU.add)
            V.scalar_tensor_tensor(a_, wtv[:, 2, :, :],
                                   bcs[:cin, 4 * col + 2:4 * col + 3], b_,
                                   op0=ALU.mult, op1=ALU.add)
            V.scalar_tensor_tensor(b_, wtv[:, 3, :, :],
                                   bcs[:cin, 4 * col + 3:4 * col + 4], a_,
                                   op0=ALU.mult, op1=ALU.add)
            agv = b_.rearrange("p (t o) -> p t o", t=9)

            zt = pz.tile([120, 2304], BF, tag="zt", bufs=S)
            zts[b] = zt
            if pad:
                hview, wview, woff = hin + 2, PW, 1
            else:
                hview, wview, woff = hin, hin, 0
            xv = cur_tiles[b][0:cin, 0:hview * wview].rearrange(
                "p (h w) -> p h w", h=hview)

            grpA, grpB = CHUNKS[i]
            psA = conv_ps.tile([124, 3 * 512], FP, tag="psA", bufs=1)
            psAv = psA[:].rearrange("p (c n) -> p c n", n=512)
            psB = conv_ps.tile([124, 2 * 512], FP, tag="psB", bufs=1)
            psBv = psB[:].rearrange("p (c n) -> p c n", n=512)

            for grp, psv in ((grpA, psAv), (grpB, psBv)):
                for ci, (y0, rows) in enumerate(grp):
                    n = rows * wout
                    for t in range(9):
                        dy, dx = divmod(t, 3)
                        rhs = xv[:, y0 + dy:y0 + dy + rows,
                                 dx + woff:dx + woff + wout]
                        nc.tensor.matmul(
                            psv[0:cout, ci, 0:n], agv[:, t, :], rhs,
                            start=(t == 0), stop=(t == 8))
            # batched evictions (strided PSUM read, contiguous zt write)
            rA = grpA[0][1]
            nA = rA * wout
            eA = 3 * nA
            A.activation(
                zt[0:cout, 0:eA].rearrange("p (c n) -> p c n", c=3),
                psAv[0:cout, :, 0:nA], AF.Identity, bias=aggb[:cout, col:col + 1],
                accum_out=sums[:cout, b:b + 1])
            if i == 3:
                # B group non-uniform: (30,10)=460, (40,6)=276
                A.activation(zt[0:cout, eA:eA + 460], psBv[0:cout, 0, 0:460],
                             AF.Identity, bias=aggb[:cout, col:col + 1],
                             accum_out=sums[:cout, S + b:S + b + 1])
                A.activation(zt[0:cout, eA + 460:eA + 460 + 276],
                             psBv[0:cout, 1, 0:276],
                             AF.Identity, bias=aggb[:cout, col:col + 1],
                             accum_out=sums[:cout, 2 * S + b:2 * S + b + 1])
            else:
                rB = grpB[0][1]
                nBc = rB * wout
                nB = len(grpB) * nBc
                A.activation(
                    zt[0:cout, eA:eA + nB].rearrange("p (c n) -> p c n",
                                                     c=len(grpB)),
                    psBv[0:cout, 0:len(grpB), 0:nBc], AF.Identity,
                    bias=aggb[:cout, col:col + 1],
                    accum_out=sums[:cout, S + b:S + b + 1])
            npix = hout * wout
            # sum of squares: V for 0-4, Scalar for 5-7
            if b < 5:
                V.scalar_tensor_tensor(
                    trashV[0:cout, 0:npix], zt[0:cout, 0:npix], 0.0,
                    zt[0:cout, 0:npix], op0=ALU.add, op1=ALU.mult,
                    accum_out=sqs[:cout, b:b + 1])
            else:
                A.activation(trashG[0:cout, 0:npix], zt[0:cout, 0:npix],
                             AF.Square, accum_out=sqs[:cout, b:b + 1])

        for i, (cin, cout, pad, hin, hout, hid) in enumerate(STAGES, 1):
            wout = hout
            # sums: evict accums (A -> col b, B -> col S+b, stage-3 second
            # B chunk -> col 2S+b); sqs: per-sample sum of squares.
            ncols = 3 * S if i == 3 else 2 * S
            sums = psm.tile([124, 3 * S], FP, tag="sums", bufs=2)
            sqs = psm.tile([124, S], FP, tag="sqs", bufs=2)

            for bi, (b0, b1) in enumerate(BATCHES):
                aggb, bcs = emit_mlp(i, bi, pooled_b[bi])
                for b in range(b0, b1):
                    emit_sample(i, b, bi, aggb, bcs, sums, sqs)

            # --- BN stats: local reduce + cross-core all-reduce ---
            stot = psm.tile([124, 1], FP, tag="stot", bufs=2)
            V.tensor_reduce(stot[0:cout, :], sums[0:cout, 0:ncols], axis=AX.X,
                            op=ALU.add)
            qtot = psm.tile([124, 1], FP, tag="qtot", bufs=2)
            V.tensor_reduce(qtot[0:cout, :], sqs[0:cout, :], axis=AX.X,
                            op=ALU.add)
            bin_t = pdram.tile([2 * cout], FP, tag=f"bnc_in{i}",
                               name=f"bnc_in{i}")
            bout_t = pdram.tile([2 * cout], FP, tag=f"bnc_out{i}",
                                name=f"bnc_out{i}", addr_space="Shared")
            dma(bin_t[0:cout], stot[0:cout, :])
            dma(bin_t[cout:2 * cout], qtot[0:cout, :])
            nc.gpsimd.collective_compute(
                "AllReduce",
                ALU.add,
                ins=[bin_t[:].opt()],
                outs=[bout_t[:].opt()],
                replica_groups=[list(range(N_CORES))],
            )
            msum = msums[(i - 1) % 2]
            dma(msum[:cout, :], bout_t[0:cout])
            msq = msqs[(i - 1) % 2]
            dma(msq[:cout, :], bout_t[cout:2 * cout])
            ntot = float(NB * hout * wout)
            mean = psm.tile([124, 1], FP, tag="mean", bufs=2)
            V.tensor_scalar(mean[0:cout, :], msum[:cout, :], 1.0 / ntot, None,
                            op0=ALU.mult)
            m2t = psm.tile([124, 1], FP, tag="m2t", bufs=2)
            V.tensor_tensor(m2t[0:cout, :], mean[0:cout, :], mean[0:cout, :],
                            op=ALU.mult)
            var = psm.tile([124, 1], FP, tag="var", bufs=2)
            V.scalar_tensor_tensor(var[0:cout, :], msq[:cout, :], 1.0 / ntot,
                                   m2t[0:cout, :], op0=ALU.mult,
                                   op1=ALU.subtract)
            std = psm.tile([124, 1], FP, tag="std", bufs=2)
            A.activation(std[0:cout, :], var[0:cout, :], AF.Sqrt,
                         bias=eps_col[0:cout, :])
            rstd = psm.tile([124, 1], FP, tag="rstd", bufs=2)
            V.reciprocal(rstd[0:cout, :], std[0:cout, :])
            gh = psm.tile([124, 1], FP, tag="gh", bufs=2)
            V.tensor_tensor(gh[0:cout, :], bng_t[i][:], rstd[0:cout, :],
                            op=ALU.mult)
            mg = psm.tile([124, 1], FP, tag="mg", bufs=2)
            V.tensor_tensor(mg[0:cout, :], mean[0:cout, :], gh[0:cout, :],
                            op=ALU.mult)
            bh = psm.tile([124, 1], FP, tag="bh", bufs=2)
            V.tensor_tensor(bh[0:cout, :], bnb_t[i][:], mg[0:cout, :],
                            op=ALU.subtract)

            # --- BN apply + relu on ScalarE (fused pooled accum) ---
            if i < 4:
                pad2 = STAGES[i][2]
                hout2 = hout
                pooled_nb = [psm.tile([124, b1 - b0], FP, tag="pooled",
                                      bufs=6, name=f"pooled_{i}_{bi}")
                             for bi, (b0, b1) in enumerate(BATCHES)]
                nxt_tiles = []
                for b in range(S):
                    bi = 0 if b < 2 else (1 if b < 4 else 2)
                    col = b - BATCHES[bi][0]
                    if pad2:
                        xt = pact.tile([120, 50 * PW], BF, tag="xt", bufs=S)
                        xv2 = xt[:].rearrange("p (h w) -> p h w", h=50)
                        G.memset(xv2[0:cout, 0, :], 0.0)
                        G.memset(xv2[0:cout, 49, :], 0.0)
                        V.memset(xv2[0:cout, 1:49, 0:2], 0.0)
                        V.memset(xv2[0:cout, 1:49, 50:52], 0.0)
                        outap = xv2[0:cout, 1:49, 2:50]
                    else:
                        xt = pact.tile([120, 2304], BF, tag="xt", bufs=S)
                        outap = xt[0:cout, 0:hout * wout]
                    A.activation(outap, zts[b][0:cout, 0:hout * wout],
                                 AF.Relu, bias=bh[0:cout, :],
                                 scale=gh[0:cout, :],
                                 accum_out=pooled_nb[bi][:cout, col:col + 1])
                    nxt_tiles.append(xt)
                cur_tiles = nxt_tiles
                pooled_b = pooled_nb
            else:
                for b in range(S):
                    A.activation(y4all[:, b * HW4:(b + 1) * HW4],
                                 zts[b][0:64, 0:HW4], AF.Relu,
                                 bias=bh[0:64, :], scale=gh[0:64, :])

        # ================= gate head =================
        y4v = y4all[:].rearrange("p (b h w) -> p b h w", b=S, h=H4)
        y4vT = y4all[:].rearrange("p (b h w) -> p b w h", b=S, h=H4)
        # plane reductions (V): sums and maxes, batched over all samples,
        # written into horizontally padded planes (band-conv inputs)
        t1all = pact.tile([64, S * 50], BF, tag="t1all", name="t1all")
        V.memset(t1all[:], 0.0)
        t1v = t1all[:].rearrange("p (b w) -> p b w", b=S)
        t2all = pact.tile([64, S * 50], BF, tag="t2all", name="t2all")
        V.memset(t2all[:], 0.0)
        t2v = t2all[:].rearrange("p (b h) -> p b h", b=S)
        with nc.allow_low_precision(reason="bf16 ZPool mean planes"):
            V.tensor_reduce(t1v[:, :, 3:47], y4vT, axis=AX.X, op=ALU.add)
            V.tensor_reduce(t2v[:, :, 3:47], y4v, axis=AX.X, op=ALU.add)
        m1all = pact.tile([64, S * 50], BF, tag="m1all", name="m1all")
        V.memset(m1all[:], 0.0)
        m1v = m1all[:].rearrange("p (b w) -> p b w", b=S)
        V.tensor_reduce(m1v[:, :, 3:47], y4vT, axis=AX.X, op=ALU.max)
        m2all = pact.tile([64, S * 50], BF, tag="m2all", name="m2all")
        V.memset(m2all[:], 0.0)
        m2v = m2all[:].rearrange("p (b h) -> p b h", b=S)
        V.tensor_reduce(m2v[:, :, 3:47], y4v, axis=AX.X, op=ALU.max)

        conv_ps_cm.__exit__(None, None, None)
        stage_ps_cm.__exit__(None, None, None)
        gate_ps_cm = tc.tile_pool(name="gateps", bufs=1, space="PSUM")
        gate_ps = gate_ps_cm.__enter__()

        # gate-3 comps per sample: channel max on GpSimd (partition
        # reduce), channel sum via K=64 ones-matmuls + ScalarE evictions;
        # both spread into the [44, S*44] band-conv layouts by DMA.
        for b in range(S):
            pb = pz.tile([64, HW4], BF, tag="parb", bufs=2)
            G.partition_all_reduce(pb[:], y4all[:, b * HW4:(b + 1) * HW4],
                                   channels=64,
                                   reduce_op=bass_isa.ReduceOp.max)
            dma(g3max_bt[b][:, 3:47], pb[0:1, :].rearrange(
                "p (h w) -> p h w", h=H4))
            mrow = pz.tile([1, HW4], BF, tag="m3row", bufs=2)
            for ci in range(4):
                psc = gate_ps.tile([1, 512], FP, tag="csps", bufs=4)
                nc.tensor.matmul(psc[:, 0:484], ones_col[0:64, :],
                                 y4all[:, b * HW4 + ci * 484:
                                       b * HW4 + (ci + 1) * 484],
                                 start=True, stop=True)
                A.activation(mrow[:, ci * 484:(ci + 1) * 484],
                             psc[0:1, 0:484], AF.Copy)
            dma(g3sum_bt[b][:, 3:47], mrow[:].rearrange(
                "p (h w) -> p h w", h=H4))

        # --- band convs: 14 batched matmuls per gate ---
        gstats = psm.tile([64, 6], FP, tag="gstats")
        V.memset(gstats[:], 0.0)
        gate_src = ((m1all, t1all, 64), (m2all, t2all, 64),
                    (g3max_bt, g3sum_bt, 44))
        gcv = []
        for g, (maxs, sums_, m_) in enumerate(gate_src):
            gp = gate_ps.tile([64, S * 44], FP, tag="gp", bufs=3)
            gpv = gp[:].rearrange("p (b n) -> p b n", b=S)
            for b in range(S):
                for idx in range(14):
                    ch, dx = divmod(idx, 7)
                    if g < 2:
                        src_ = maxs if ch == 0 else sums_
                        rhs = src_[0:m_, b * 50 + dx:b * 50 + dx + 44]
                    else:
                        srcb = maxs[b] if ch == 0 else sums_[b]
                        rhs = srcb[:, dx:dx + 44]
                    nc.tensor.matmul(
                        gpv[0:m_, b, :],
                        gb_t[g][:, idx * m_:(idx + 1) * m_],
                        rhs,
                        start=(idx == 0), stop=(idx == 13))
            cvall = psm.tile([64, S * 44], BF, tag=f"gcva{g}", name=f"gcva{g}")
            A.activation(cvall[0:m_, :], gp[0:m_, :], AF.Copy,
                         accum_out=gstats[:m_, 2 * g:2 * g + 1])
            V.scalar_tensor_tensor(
                trashV[0:m_, 0:S * 44], cvall[0:m_, :], 0.0, cvall[0:m_, :],
                op0=ALU.add, op1=ALU.mult,
                accum_out=gstats[:m_, 2 * g + 1:2 * g + 2])
            gcv.append(cvall)
        # partition-sum of the 6 stat columns in one matmul
        sps = gate_ps.tile([6, 1], FP, tag="spsum", bufs=1)
        nc.tensor.matmul(sps[:], gstats[:], ones_colf[0:64, :],
                         start=True, stop=True)
        s6 = psm.tile([6, 1], FP, tag="s6")
        A.activation(s6[:], sps[:], AF.Copy)

        # --- one AllReduce for all three gate BNs ---
        gbin = pdram.tile([6], FP, tag="gbin")
        gbout = pdram.tile([6], FP, tag="gbout", addr_space="Shared")
        dma(gbin[:], s6[:])
        nc.gpsimd.collective_compute(
            "AllReduce",
            ALU.add,
            ins=[gbin[:].opt()],
            outs=[gbout[:].opt()],
            replica_groups=[list(range(N_CORES))],
        )
        dma(gtot_in[:], gbout[:])
        ghbh = psm.tile([1, 6], FP, tag="ghbh")
        planes_n = [64 * H4, 64 * H4, H4 * H4]
        for g in range(3):
            n = float(NB * planes_n[g])
            gmean = psm.tile([1, 1], FP, tag="gmean", bufs=3)
            V.tensor_scalar(gmean[:], gtot_in[:, 2 * g:2 * g + 1], 1.0 / n,
                            None, op0=ALU.mult)
            gm2 = psm.tile([1, 1], FP, tag="gm2", bufs=3)
            V.tensor_tensor(gm2[:], gmean[:], gmean[:], op=ALU.mult)
            gvar = psm.tile([1, 1], FP, tag="gvar", bufs=3)
            V.scalar_tensor_tensor(gvar[:], gtot_in[:, 2 * g + 1:2 * g + 2],
                                   1.0 / n, gm2[:], op0=ALU.mult,
                                   op1=ALU.subtract)
            gstd = psm.tile([1, 1], FP, tag="gstd", bufs=3)
            A.activation(gstd[:], gvar[:], AF.Sqrt, bias=eps_col[0:1, :])
            grstd = psm.tile([1, 1], FP, tag="grstd", bufs=3)
            V.reciprocal(grstd[:], gstd[:])
            V.tensor_tensor(ghbh[:, 2 * g:2 * g + 1],
                            gbn_t[0:1, 2 * g:2 * g + 1], grstd[:],
                            op=ALU.mult)
            gmg = psm.tile([1, 1], FP, tag="gmg", bufs=3)
            V.tensor_tensor(gmg[:], gmean[:], ghbh[:, 2 * g:2 * g + 1],
                            op=ALU.mult)
            V.tensor_tensor(ghbh[:, 2 * g + 1:2 * g + 2],
                            gbn_t[0:1, 2 * g + 1:2 * g + 2], gmg[:],
                            op=ALU.subtract)
        gb64 = psm.tile([64, 6], FP, tag="gb64", name="gb64")
        G.partition_broadcast(gb64[:], ghbh[:], channels=64)

        # --- sigmoids + contributions: out1 = c1 + c2 + c3 ---
        gate_ps_cm.__exit__(None, None, None)
        c123 = psm.tile([64, 3 * S], FP, tag="c123")
        inv3hw = 1.0 / (3.0 * HW4)
        bc3_ps_cm = tc.tile_pool(name="bc3ps", bufs=1, space="PSUM")
        bc3_ps = bc3_ps_cm.__enter__()
        sgs = []
        for g, m_ in ((0, 64), (1, 64), (2, 44)):
            sg = psm.tile([64, S * 44], BF, tag=f"sga{g}", name=f"sga{g}")
            A.activation(sg[0:m_, :], gcv[g][0:m_, :], AF.Sigmoid,
                         bias=gb64[:m_, 2 * g + 1:2 * g + 2],
                         scale=gb64[:m_, 2 * g:2 * g + 1])
            sgs.append(sg)
        tr44 = psm.tile([64, 44], FP, tag="tr44")
        tr44b = psm.tile([64, 44], FP, tag="tr44b")
        for b in range(S):
            V.scalar_tensor_tensor(
                tr44[:], t1all[:, b * 50 + 3:b * 50 + 47], inv3hw,
                sgs[0][:, b * 44:(b + 1) * 44],
                op0=ALU.mult, op1=ALU.mult, accum_out=c123[:, b:b + 1])
            V.scalar_tensor_tensor(
                tr44b[:], t2all[:, b * 50 + 3:b * 50 + 47], inv3hw,
                sgs[1][:, b * 44:(b + 1) * 44],
                op0=ALU.mult, op1=ALU.mult,
                accum_out=c123[:, S + b:S + b + 1])
            s3row = s3rows[b % 2]
            dma(s3row[:], sgs[2][0:44, b * 44:(b + 1) * 44])
            bc3 = bc3_ps.tile([64, 4 * 512], FP, tag="bc3", bufs=1)
            bc3v = bc3[:].rearrange("p (c n) -> p c n", n=512)
            for ci in range(4):
                nc.tensor.matmul(
                    bc3v[:, ci, 0:484],
                    ones_row_bf[0:1, 0:64],
                    s3row[:, ci * 484:(ci + 1) * 484],
                    start=True, stop=True,
                )
            bc3sb = pz.tile([64, HW4], BF, tag="g64tmp", bufs=2)
            A.activation(bc3sb[:], bc3v[:, :, 0:484], AF.Copy)
            V.scalar_tensor_tensor(
                trashV[0:64, 0:HW4], y4all[:, b * HW4:(b + 1) * HW4],
                inv3hw, bc3sb[:], op0=ALU.mult, op1=ALU.mult,
                accum_out=c123[:, 2 * S + b:2 * S + b + 1])
        bc3_ps_cm.__exit__(None, None, None)

        o1a = psm.tile([64, S], FP, tag="o1a")
        V.tensor_tensor(o1a[:], c123[:, 0:S], c123[:, S:2 * S], op=ALU.add)
        o1sb = psm.tile([64, S], FP, tag="o1sb")
        V.tensor_tensor(o1sb[:], o1a[:], c123[:, 2 * S:3 * S], op=ALU.add)
        dma(o1o[:, :], o1sb[:])

        pz_cm.__exit__(None, None, None)
        pwt_cm.__exit__(None, None, None)
        est.close()

    nc.compile()
    return nc


def prep_in_maps(inputs):
    f32 = np.float32
    bf16 = mybir.dt.np(BF)
    x = np.ascontiguousarray(np.asarray(inputs["x"], f32))
    common = {}
    for i, (cin, cout, pad, hin, hout, hid) in enumerate(STAGES, 1):
        W = np.asarray(inputs[f"d{i}_W"], f32)  # [4,cout,cin,3,3]
        common[f"wt{i}"] = np.ascontiguousarray(
            W.reshape(4, cout, cin, 9).transpose(2, 0, 3, 1).reshape(cin, 36 * cout)
        ).astype(bf16)
        common[f"wb{i}"] = np.ascontiguousarray(np.asarray(inputs[f"d{i}_b"], f32))
        a1 = np.asarray(inputs[f"d{i}_a1w"], f32)
        common[f"a1w{i}"] = np.ascontiguousarray(a1.T / float(hin * hin))
        common[f"a2w{i}"] = np.ascontiguousarray(np.concatenate(
            [np.asarray(inputs[f"d{i}_a2w"], f32).T,
             np.asarray(inputs[f"d{i}_a2b"], f32)[None, :]], axis=0))
        common[f"bng{i}"] = np.ascontiguousarray(
            np.asarray(inputs[f"bn{i}_g"], f32)[:, None])
        common[f"bnb{i}"] = np.ascontiguousarray(
            np.asarray(inputs[f"bn{i}_b"], f32)[:, None])
    common["fc3w"] = np.ascontiguousarray(
        np.asarray(inputs["fc3_w"], f32).T / float(48 * 48))
    common["fc3b"] = np.ascontiguousarray(
        np.tile(np.asarray(inputs["fc3_b"], f32)[None, :], (S, 1)))
    # gate order: (cw: pool over H, len 44), (hc: pool over W, len 44),
    # (hw: pool over C, len 64); mean channel folded into the conv weight.
    # Each 7x7 conv becomes 14 accumulated matmuls whose stationary operands
    # are constant band matrices B[yy, y] = k[ch, yy - y + 3, dx].
    for g, (name, plen, kk) in enumerate(
            (("cw", 44.0, 64), ("hc", 44.0, 64), ("hw", 64.0, 44))):
        w = np.asarray(inputs[f"{name}_w"], f32).copy()  # [1,2,7,7]
        if name == "hc":
            w = np.ascontiguousarray(w.transpose(0, 1, 3, 2))
        w[0, 1] /= plen
        band = np.zeros((kk, 14 * kk), f32)
        for ch in range(2):
            for dx in range(7):
                col0 = (ch * 7 + dx) * kk
                for dv in range(7):
                    vals = w[0, ch, dv, dx]
                    for y in range(kk):
                        yy = y + dv - 3
                        if 0 <= yy < kk:
                            band[yy, col0 + y] = vals
        common[f"gb{g}"] = np.ascontiguousarray(band).astype(bf16)
    common["gbn"] = np.ascontiguousarray(np.array(
        [[np.asarray(inputs["cw_g"]).reshape(-1)[0],
          np.asarray(inputs["cw_b"]).reshape(-1)[0],
          np.asarray(inputs["hc_g"]).reshape(-1)[0],
          np.asarray(inputs["hc_b"]).reshape(-1)[0],
          np.asarray(inputs["hw_g"]).reshape(-1)[0],
          np.asarray(inputs["hw_b"]).reshape(-1)[0]]], f32))
    common["ident"] = np.eye(16, dtype=f32)

    xpad = np.zeros((NB, 100, 50, PW), f32)
    xpad[:, :, 1:49, 2:50] = x
    xpad = xpad.reshape(NB, 100, 50 * PW).astype(bf16)
    in_maps = []
    for c in range(N_CORES):
        m = dict(common)
        m["x"] = np.ascontiguousarray(xpad[c * S:(c + 1) * S])
        in_maps.append(m)
    return in_maps


_NC_CACHE = None
LAST_RESULTS = None


def kernel(**inputs):
    global _NC_CACHE, LAST_RESULTS
    import os

    if _NC_CACHE is None:
        _NC_CACHE = build_nc()
    nc = _NC_CACHE
    in_maps = prep_in_maps(inputs)
    trace = bool(int(os.environ.get("KERNEL_TRACE", "0")))
    res = run_bass_kernel_spmd(
        nc, in_maps, core_ids=list(range(N_CORES)), trace=trace
    )
    LAST_RESULTS = res
    x1 = np.concatenate([res.results[c]["x1o"] for c in range(N_CORES)], axis=0)
    out1 = np.concatenate(
        [res.results[c]["o1o"].T for c in range(N_CORES)], axis=0)
    return x1.astype(np.float32), out1.astype(np.float32)
